# revision 1
# baseline (speedup 1.0000x reference)
"""HardNegTripletMarginLoss on 8 Trainium2 NeuronCores (Bass/Tile).

Strategy (anchors row-sharded across 8 cores, embeddings replicated):
  - Host: normalize rows (as reference), stable-sort rows by label, and give
    each core a column-ROTATED copy of Xn^T (quantized to bf16: full PE
    rate -- 2x f32r -- and half the DMA/SBUF) so the core's own anchor
    block sits at local columns [0, 1024). With sorted labels every anchor's
    same-label columns fall inside 2 statically-known 512-wide column tiles
    per anchor block -- identical tile indices on every core, so one SPMD
    program serves all 8 cores.
  - Device per 128-anchor block: 16 column-tiles of 512 packed into 8 PSUM
    containers of [128,1024] (the 2 masked/same-label tiles share container
    0, which gets += 16*same via a one-hot matmul):
      * exact containers: DVE tensor_reduce min straight from PSUM.
      * soft containers: ACT exp-accumulate sum(exp(-T*(v - r_mn))) straight
        from PSUM (softmin; r_mn is a host-sampled per-anchor reference that
        keeps the exponent in range; lifted same-label entries vanish).
      * hardest-positive: ACT exp-accumulate sum(exp(+T*(v - r_mx))) over
        container 0 (softmax; non-lifted entries vanish; r_mx is a
        host-sampled positive max so >= 1 term has exponent ~0).
    This splits the 8.4M-element/core eviction between DVE (~123G elem/s)
    and ACT (~154G elem/s) with no PSUM->SBUF copies and no cross-engine
    data dependencies; exact/soft issue is interleaved so both engines and
    the PE run concurrently through the 4-deep PSUM container ring.
  - Host: ln()/T finals, d2 = s_i + s_j - 2G with s_j ~= 1, sqrt, relu,
    AvgNonZero reduction, plus exact re-repair of any overflowed anchors
    (tiny, O(N) work).

This walrus build rejects instructions carrying >1 sync wait, so
Bass.to_json_bytes is wrapped to split multi-wait instructions into
single-wait Drain carriers on the same engine.
"""

import json
import os
import sys
import types
import ctypes

for _p in ("/opt/trn_rl_repo", "/root/.axon_site/_ro/trn_rl_repo"):
    if os.path.isdir(_p) and _p not in sys.path:
        sys.path.append(_p)

import numpy as np
import ml_dtypes
import concourse.bass as bass
import concourse.tile as tile
from concourse import mybir
from concourse.bass_utils import run_bass_kernel_spmd
from contextlib import ExitStack

P = 128
N = 8192
D = 128
NCORES = 8
M = N // NCORES            # anchors per core
NBLK = M // P              # anchor blocks per core
TW = 512                   # matmul moving-dim tile / half-container
SC = 1024                  # PSUM container width (2 banks)
BIG = 16.0
MARGIN = 0.05
TSOFT = 64.0               # softmin/softmax temperature (in d^2 units)
F32 = mybir.dt.float32
BF16 = mybir.dt.bfloat16
BF16NP = ml_dtypes.bfloat16

# masked (diagonal-window) global 512-col tile indices per anchor block l
MASKED_TILES = {0: (15, 0), 1: (15, 0), 2: (0, 1), 3: (0, 1),
                4: (0, 1), 5: (0, 1), 6: (1, 2), 7: (1, 2)}
# column slab layout of the one-hot cols input: tile -> slab slot
OH_SLOT = {15: 0, 0: 1, 1: 2, 2: 3}

# number of exact (DVE) containers per block; the rest (8-e) are soft (ACT)
BLOCK_E = (5, 5, 5, 5, 5, 5, 5, 5)


def _block_plan(l):
    """containers[i] = (tile_a, tile_b); container 0 holds the masked pair.
    Returns (containers, exact_idx, soft_idx, issue_order)."""
    mt = sorted(MASKED_TILES[l])
    rest = [t for t in range(16) if t not in mt]
    containers = [tuple(mt)] + [(rest[2 * i], rest[2 * i + 1]) for i in range(7)]
    e = BLOCK_E[l]
    soft = list(range(1, 8 - e + 1))        # c1..c(8-e)
    exact = [0] + list(range(8 - e + 1, 8))  # c0 + tail
    # interleave exact/soft; keep the expensive masked container (c0, 4
    # matmuls + 2 engine reads) out of the block-boundary slot
    assert e == 5, "issue order below assumes e=5"
    # soft containers early (ACT never matmul-starved); DVE's exact tail
    # covers the block boundary while the PE fills the next block
    order = [4, 1, 0, 2, 3, 5, 6, 7]
    return containers, exact, soft, order


# output layout: per block 5 exact-min cols | 4 soft-sum cols | 1 msk col
OUT_W = NBLK * 5 + NBLK * 4 + NBLK

LAST_RESULTS = None        # BassKernelResults of the most recent run (for test.py)


def _install_wait_split_patch():
    if getattr(bass.Bass, "_wait_split_patched", False):
        return
    orig = bass.Bass.to_json_bytes

    def patched(self):
        raw = orig(self)
        d = json.loads(raw)
        changed = False
        for fn in d.get("functions", []):
            for blk in fn.get("blocks", []):
                out, k = [], 0
                for ins in blk.get("instructions", []):
                    si = ins.get("sync_info") or {}
                    waits = si.get("on_wait") or []
                    if len(waits) > 1:
                        changed = True
                        for w in waits[:-1]:
                            k += 1
                            out.append({
                                "name": f"{ins['name']}-sw{k}",
                                "opcode": "Drain",
                                "engine": ins["engine"],
                                "ins": [],
                                "outs": [],
                                "is_reset_sema": False,
                                "debug": ins.get("debug", 0),
                                "sync_info": {"on_wait": [w], "on_update": []},
                            })
                        si["on_wait"] = [waits[-1]]
                    out.append(ins)
                blk["instructions"] = out
        return json.dumps(d).encode() if changed else raw

    bass.Bass.to_json_bytes = patched
    bass.Bass._wait_split_patched = True


def _ensure_ntff_hook():
    """Best-effort: restore the axon NTFF profile hook this image dropped."""
    if "antenv.axon_hooks" in sys.modules:
        return
    try:
        lib = ctypes.CDLL("/opt/axon/libaxon_pjrt.so")
        if not hasattr(lib, "axon_start_nrt_profile"):
            return
        from trn_agent_boot.trn_boot import _ntff_profile_via_ctypes
        hook = _ntff_profile_via_ctypes("/opt/axon/libaxon_pjrt.so")
        mod = types.ModuleType("antenv.axon_hooks")
        mod._hook = hook
        mod.get_axon_ntff_profile_hook = lambda: mod._hook
        mod.set_axon_ntff_profile_hook = lambda h: setattr(mod, "_hook", h)
        sys.modules["antenv.axon_hooks"] = mod
        import antenv
        antenv.axon_hooks = mod
    except Exception:
        pass


def _build_nc():
    nc = bass.Bass("TRN2", target_bir_lowering=False, debug=False)
    xt_d = nc.dram_tensor("xt", [P, N], BF16, kind="ExternalInput")
    xa_d = nc.dram_tensor("xm2a", [P, M], BF16, kind="ExternalInput")
    ohc_d = nc.dram_tensor("ohc", [64, 4 * TW], BF16, kind="ExternalInput")
    oha_d = nc.dram_tensor("oha", [64, M], BF16, kind="ExternalInput")
    bmn_d = nc.dram_tensor("bmn", [P, NBLK], F32, kind="ExternalInput")
    bmx_d = nc.dram_tensor("bmx", [P, NBLK], F32, kind="ExternalInput")
    out_d = nc.dram_tensor("per_out", [P, OUT_W], F32, kind="ExternalOutput")

    with tile.TileContext(nc) as tc, ExitStack() as ctx:
        xpool = ctx.enter_context(tc.tile_pool(name="xt", bufs=8))
        inpool = ctx.enter_context(tc.tile_pool(name="ins", bufs=1))
        ppool = ctx.enter_context(tc.tile_pool(name="psum", bufs=4, space="PSUM"))
        scpool = ctx.enter_context(tc.tile_pool(name="scr", bufs=2))
        accpool = ctx.enter_context(tc.tile_pool(name="acc", bufs=1))
        fpool = ctx.enter_context(tc.tile_pool(name="fin", bufs=2))

        # preload the exp activation table during the input DMAs so the first
        # soft container doesn't pay the ~2.7us table load
        warm = fpool.tile([P, 1], F32, tag="warm")
        nc.vector.memset(warm[:], 0.0)
        nc.scalar.activation(warm[:], warm[:], mybir.ActivationFunctionType.Exp)

        # DMA issue order = descriptor order on the 16 rings = arrival
        # order; issue exactly what block 0 consumes first
        xa = inpool.tile([P, M], BF16, tag="xa")
        nc.sync.dma_start(xa[:], xa_d.ap()[:, :])
        xch = [None] * 8
        def _dma_piece(ch):
            t = xpool.tile([P, SC], BF16, tag="xch")
            nc.sync.dma_start(t[:], xt_d.ap()[:, ch * SC:(ch + 1) * SC])
            xch[ch] = t
        for ch in (3, 4):          # container c4 = tiles (7, 8)
            _dma_piece(ch)
        bmn = inpool.tile([P, NBLK], F32, tag="bmn")
        nc.sync.dma_start(bmn[:], bmn_d.ap()[:, :])
        oha = inpool.tile([64, M], BF16, tag="oha")
        nc.sync.dma_start(oha[:], oha_d.ap()[:, :])
        ohc = inpool.tile([64, 4 * TW], BF16, tag="ohc")
        nc.sync.dma_start(ohc[:], ohc_d.ap()[:, :])
        bmx = inpool.tile([P, NBLK], F32, tag="bmx")
        nc.sync.dma_start(bmx[:], bmx_d.ap()[:, :])
        for ch in (0, 1, 7, 2, 5, 6):
            _dma_piece(ch)

        def xcol(tg):
            ch, off = tg // 2, (tg % 2) * TW
            return xch[ch][:, off:off + TW]

        acc_e = accpool.tile([P, 5 * NBLK], F32, tag="acc_e", name="acc_e")
        ssm = accpool.tile([P, 4 * NBLK], F32, tag="ssm", name="ssm")
        msk = accpool.tile([P, NBLK], F32, tag="msk", name="msk")

        def mm_container(ps, l, pair):
            """matmul the two global tiles of a container into ps halves."""
            lhsT = xa[:, l * P:(l + 1) * P]
            for half, tg in enumerate(pair):
                m = tg in MASKED_TILES[l]
                nc.tensor.matmul(
                    ps[:, half * TW:(half + 1) * TW],
                    lhsT=lhsT, rhs=xcol(tg), start=True, stop=not m)
                if m:
                    nc.tensor.matmul(
                        ps[:, half * TW:(half + 1) * TW],
                        lhsT=oha[:, l * P:(l + 1) * P],
                        rhs=ohc[:, OH_SLOT[tg] * TW:(OH_SLOT[tg] + 1) * TW],
                        start=False, stop=True)

        for l in range(NBLK):
            containers, exact, soft, order = _block_plan(l)
            for ci in order:
                ps = ppool.tile([P, SC], F32, tag="ps")
                mm_container(ps, l, containers[ci])
                if ci == 0:
                    # hardest-positive softmax over the masked container
                    sm = scpool.tile([P, SC], F32, tag="sm")
                    nc.scalar.activation(
                        sm[:], ps[:], mybir.ActivationFunctionType.Exp,
                        bias=bmx[:, l:l + 1], scale=TSOFT,
                        accum_out=msk[:, l:l + 1])
                if ci in exact:
                    k = exact.index(ci)
                    nc.vector.tensor_reduce(
                        acc_e[:, 5 * l + k:5 * l + k + 1], ps[:],
                        op=mybir.AluOpType.min, axis=mybir.AxisListType.X)
                else:
                    j = soft.index(ci)
                    sb = scpool.tile([P, SC], F32, tag="sb")
                    nc.scalar.activation(
                        sb[:], ps[:], mybir.ActivationFunctionType.Exp,
                        bias=bmn[:, l:l + 1], scale=-TSOFT,
                        accum_out=ssm[:, 4 * l + j:4 * l + j + 1])

        nc.sync.dma_start(out_d.ap()[:, 0:5 * NBLK], acc_e[:])
        nc.sync.dma_start(out_d.ap()[:, 5 * NBLK:9 * NBLK], ssm[:])
        nc.sync.dma_start(out_d.ap()[:, 9 * NBLK:10 * NBLK], msk[:])
    return nc


def _reference_fallback(embeddings, labels):
    x = embeddings / np.maximum(
        np.sqrt((embeddings * embeddings).sum(1, keepdims=True)), 1e-12)
    sq = (x * x).sum(1)
    d2 = sq[:, None] + sq[None, :] - 2.0 * (x @ x.T)
    dist = np.sqrt(np.maximum(d2, 0.0))
    same = labels[:, None] == labels[None, :]
    eye = np.eye(len(labels), dtype=bool)
    pos, neg = same & ~eye, ~same
    d_ap = np.where(pos, dist, -np.inf).max(1)
    d_an = np.where(neg, dist, np.inf).min(1)
    valid = pos.any(1) & neg.any(1)
    per = np.maximum(d_ap - d_an + MARGIN, 0.0)
    per = np.where(valid, per, 0.0)
    nz = (per > 0).sum()
    return np.float32(per.sum() / max(nz, 1)) if nz > 0 else np.float32(0.0)


def kernel(embeddings: np.ndarray, labels: np.ndarray) -> np.ndarray:
    global LAST_RESULTS
    emb = np.asarray(embeddings, dtype=np.float32)
    lab = np.asarray(labels).reshape(-1)

    counts = np.bincount(lab.astype(np.int64) - lab.min())
    if emb.shape != (N, D) or counts.max() > 256 or len(np.unique(lab)) < 2:
        return np.array(_reference_fallback(emb, lab), dtype=np.float32)

    norms = np.sqrt((emb * emb).sum(1, keepdims=True, dtype=np.float32))
    xn = emb / np.maximum(norms, np.float32(1e-12))
    s = (xn * xn).sum(1, dtype=np.float32)

    perm = np.argsort(lab, kind="stable")
    xs = xn[perm]
    ls = lab[perm]
    ss = s[perm]

    uniq = np.unique(ls)
    code = np.searchsorted(uniq, ls).astype(np.int64)
    assert len(uniq) <= 64

    # fp8 operand planes (device sees these exact values)
    xs8 = xs.astype(BF16NP)
    xs8f = xs8.astype(np.float32)
    xa8 = (-2.0 * xs).astype(BF16NP)
    xa8f = xa8.astype(np.float32)

    rng = np.random.default_rng(7)
    # hardest-positive softmax reference: sampled same-label max (lifted)
    r_mx = np.empty(N, dtype=np.float32)
    for g in range(len(uniq)):
        idx = np.where(code == g)[0]
        samp = rng.choice(idx, size=min(16, len(idx)), replace=False)
        gm = xs8f[idx] @ xs8f[samp].T
        r_mx[idx] = (-2.0 * gm).max(1) + np.float32(BIG)

    # per (core, block) soft columns from the static plan
    soft_cols_cl = {}
    for l in range(NBLK):
        containers, exact, soft, _ = _block_plan(l)
        cols = []
        for ci in soft:
            for tg in containers[ci]:
                cols.extend(range(tg * TW, (tg + 1) * TW))
        soft_cols_cl[l] = np.array(cols)

    # softmin reference: sampled min over the block's soft columns
    r_mn = np.empty(N, dtype=np.float32)
    for c in range(NCORES):
        lo = c * M
        rot = np.roll(np.arange(N), -lo)
        for l in range(NBLK):
            rows = np.arange(lo + l * P, lo + (l + 1) * P)
            sc = soft_cols_cl[l]
            samp = rng.choice(sc, size=96, replace=False)
            srows = rot[samp]
            v = -2.0 * (xs8f[rows] @ xs8f[srows].T)
            v += BIG * (ls[rows][:, None] == ls[srows][None, :])
            r_mn[rows] = v.min(1)

    _install_wait_split_patch()
    _ensure_ntff_hook()
    nc = _build_nc()

    in_maps = []
    for c in range(NCORES):
        lo = c * M
        rot = np.roll(np.arange(N), -lo)
        xt = np.ascontiguousarray(xs8[rot].T)
        xm2a = np.ascontiguousarray(xa8[lo:lo + M].T)
        slab = np.concatenate([rot[N - TW:], rot[:3 * TW]])
        ohc = (code[slab][None, :] == np.arange(64)[:, None]).astype(BF16NP)
        oha = (BIG * (code[lo:lo + M][None, :] == np.arange(64)[:, None])).astype(BF16NP)
        bmn = np.ascontiguousarray(
            (TSOFT * r_mn[lo:lo + M]).reshape(NBLK, P).T.astype(np.float32))
        bmx = np.ascontiguousarray(
            (-TSOFT * r_mx[lo:lo + M]).reshape(NBLK, P).T.astype(np.float32))
        in_maps.append({"xt": xt, "xm2a": xm2a, "ohc": ohc, "oha": oha,
                        "bmn": bmn, "bmx": bmx})

    res = run_bass_kernel_spmd(nc, in_maps, core_ids=list(range(NCORES)))
    LAST_RESULTS = res

    d_ap_all = np.empty(N, dtype=np.float64)
    d_an_all = np.empty(N, dtype=np.float64)
    bad = []
    for c in range(NCORES):
        o = np.asarray(res.results[c]["per_out"], dtype=np.float64)
        lo = c * M
        for l in range(NBLK):
            rows = np.arange(lo + l * P, lo + (l + 1) * P)
            e = BLOCK_E[l]
            m_exact = o[:, 5 * l:5 * l + e].min(1)
            ssum = o[:, 5 * NBLK + 4 * l:5 * NBLK + 4 * l + (8 - e)].sum(1)
            msum = o[:, 9 * NBLK + l]
            ok = np.isfinite(ssum) & np.isfinite(msum) & (msum > 0)
            bad.extend(rows[~ok])
            mn_soft = r_mn[rows] - np.log(np.maximum(ssum, 1e-30)) / TSOFT
            mn = np.minimum(m_exact, mn_soft)
            mx = r_mx[rows] + np.log(np.maximum(msum, 1e-30)) / TSOFT
            s_i = ss[rows]
            d_an_all[rows] = np.sqrt(np.maximum(s_i + 1.0 + mn, 0.0))
            d_ap_all[rows] = np.sqrt(np.maximum(s_i + 1.0 + mx - BIG, 0.0))

    if bad:
        # overflowed/degenerate anchors: recompute exactly on host (rare)
        for i in bad:
            g = xs8f @ xs8f[i]
            d2 = ss + ss[i] - 2.0 * g
            d = np.sqrt(np.maximum(d2, 0.0))
            samel = ls == ls[i]
            posm = samel.copy()
            posm[i] = False
            d_ap_all[i] = d[posm].max() if posm.any() else 0.0
            d_an_all[i] = d[~samel].min()

    per = np.maximum(d_ap_all - d_an_all + MARGIN, 0.0)
    nz = int((per > 0).sum())
    if nz == 0:
        return np.array(0.0, dtype=np.float32)
    return np.array(np.float32(per.sum() / nz), dtype=np.float32)


if __name__ == "__main__":
    from concourse import bass_utils
    import tempfile
    _install_wait_split_patch()
    nc = _build_nc()
    td = tempfile.mkdtemp(prefix="tripletk_")
    print(bass_utils.compile_bass_kernel(nc, td))

